# revision 2
# baseline (speedup 1.0000x reference)
"""Attention pooling kernel for Trainium2 (8 NeuronCores).

Computes: scores = E @ q; w = softmax(scores); out = w @ E
for E [N=2097152, 64] fp32, q [64] fp32.

Strategy (per core, N/8 = 262144 rows), fp16 wire format (halves HBM
traffic vs fp32; measured rel err ~1.4e-3 vs the 2e-2 gate):
  - Host packs the core's row-shard into a "2-row-packed transposed"
    fp16 layout Epack [128, C=131072]: partition k = p*64 + d holds
    E[2n + p, d] at column n.  DMA is contiguous per partition.
  - Scores via TensorE fp16 matmuls (1 cyc/row) with stationary
    qmat [128, 128], qmat[k, m] = q[k % 64] if (k // 64 == m // 64)
    else 0.  PSUM out[m, n] = s(2n + m//64): scores land replicated
    across the 64 partitions of each parity half, which broadcasts the
    weights for the weighted-sum multiply for free.
  - ACT: w = exp(scores - C) PSUM->SBUF in bf16, fused accum_out
    writes per-group sumexp partials into slot columns (no per-tile
    folding on the critical path).
  - DVE: fused scalar_tensor_tensor (junk = Epack * w, accum_out =
    per-partition sum) accumulates the weighted sum into slot columns:
    partition k = (p, d) gets sum_n E[2n+p, d] * w(2n+p).
  - Final: two tensor_reduce ops fold the slot columns; host sums
    cores/parities and divides by sumexp.  The shift C (from q alone)
    cancels in the division.
"""

import sys

sys.path.insert(0, "/opt/trn_rl_repo")

import numpy as np

N_TOTAL = 2097152
D = 64
N_CORES = 8
N_PER_CORE = N_TOTAL // N_CORES          # 262144
COLS_PER_CORE = N_PER_CORE // 2          # 131072 packed columns
MM_N = 512                               # matmul free dim (one PSUM bank)
DMA_COLS = 8192                          # columns per DMA tile
EXP_COLS = 2048                          # columns per exp op (4 PSUM banks)

_compiled = {}


def _build_nc(n_cols, dma_cols, exp_cols):
    import concourse.bacc as bacc
    import concourse.bass as bass
    import concourse.mybir as mybir
    import concourse.tile as tile

    fp32 = mybir.dt.float32
    fp16 = mybir.dt.float16
    bf16 = mybir.dt.bfloat16

    nc = bacc.Bacc()
    ep_dram = nc.declare_dram_parameter("epack", [128, n_cols], fp16, isOutput=False)
    qmat_dram = nc.declare_dram_parameter("qmat", [128, 128], fp16, isOutput=False)
    cshift_dram = nc.declare_dram_parameter("cshift", [128, 1], fp32, isOutput=False)
    out_dram = nc.declare_dram_parameter("out", [128, 2], fp32, isOutput=True)

    n_tiles = n_cols // dma_cols
    groups = dma_cols // exp_cols
    n_acts = n_tiles * groups            # total exp instructions

    with tile.TileContext(nc) as tc:
        with (
            tc.tile_pool(name="const", bufs=1) as const_pool,
            tc.tile_pool(name="ep", bufs=3) as ep_pool,
            tc.tile_pool(name="w", bufs=2) as w_pool,
            tc.tile_pool(name="junk", bufs=1) as junk_pool,
            tc.tile_pool(name="acc", bufs=1) as acc_pool,
            tc.tile_pool(name="psum", bufs=2, space=bass.MemorySpace.PSUM) as psum_pool,
        ):
            qmat = const_pool.tile([128, 128], fp16, tag="qmat")
            cshift = const_pool.tile([128, 1], fp32, tag="cshift")
            nc.sync.dma_start(qmat[:], qmat_dram[:])
            nc.sync.dma_start(cshift[:], cshift_dram[:])

            # slot-column accumulators: one column per producing instruction
            master_se = acc_pool.tile([128, n_acts], fp32, tag="master_se")
            master_aw = acc_pool.tile([128, n_tiles], fp32, tag="master_aw")

            for t in range(n_tiles):
                ep = ep_pool.tile([128, dma_cols], fp16, tag="ep")
                nc.sync.dma_start(ep[:], ep_dram[:, t * dma_cols:(t + 1) * dma_cols])

                w_sb = w_pool.tile([128, dma_cols], bf16, tag="w")
                for g in range(groups):
                    lo = g * exp_cols
                    ps = psum_pool.tile([128, exp_cols], fp32, tag="ps")
                    for k in range(exp_cols // MM_N):
                        nc.tensor.matmul(
                            ps[:, k * MM_N:(k + 1) * MM_N],
                            qmat[:],
                            ep[:, lo + k * MM_N:lo + (k + 1) * MM_N],
                            start=True,
                            stop=True,
                        )
                    # w = exp(scores - C); accum gives per-group sumexp
                    ga = t * groups + g
                    nc.scalar.activation(
                        w_sb[:, lo:lo + exp_cols],
                        ps[:],
                        mybir.ActivationFunctionType.Exp,
                        bias=cshift[:, 0:1],
                        scale=1.0,
                        accum_out=master_se[:, ga:ga + 1],
                    )
                junk = junk_pool.tile([128, dma_cols], fp16, tag="junk")
                nc.vector.scalar_tensor_tensor(
                    junk[:],
                    ep[:],
                    1.0,
                    w_sb[:],
                    op0=mybir.AluOpType.mult,
                    op1=mybir.AluOpType.mult,
                    accum_out=master_aw[:, t:t + 1],
                )

            res = acc_pool.tile([128, 2], fp32, tag="res")
            nc.vector.tensor_reduce(
                res[:, 0:1], master_aw[:], axis=mybir.AxisListType.X,
                op=mybir.AluOpType.add,
            )
            nc.vector.tensor_reduce(
                res[:, 1:2], master_se[:], axis=mybir.AxisListType.X,
                op=mybir.AluOpType.add,
            )
            nc.sync.dma_start(out_dram[:], res[:])

    nc.compile()
    return nc


def _pack_core(e_core):
    # [Nc, 64] -> [n, p, d] -> [(p, d), n] in fp16
    nc_rows = e_core.shape[0]
    return np.ascontiguousarray(
        e_core.reshape(nc_rows // 2, 2, D).transpose(1, 2, 0).reshape(128, nc_rows // 2)
    ).astype(np.float16)


def _make_consts(query):
    c_shift = float(6.0 * np.linalg.norm(query.astype(np.float64)))
    qmat = np.zeros((128, 128), dtype=np.float16)
    q16 = query.astype(np.float16)
    qmat[0:64, 0:64] = q16[:, None]
    qmat[64:128, 64:128] = q16[:, None]
    cshift = np.full((128, 1), -c_shift, dtype=np.float32)
    return qmat, cshift


def kernel(embeddings, query):
    from concourse.bass_utils import run_bass_kernel_spmd

    embeddings = np.asarray(embeddings, dtype=np.float32)
    query = np.asarray(query, dtype=np.float32)

    key = (COLS_PER_CORE, DMA_COLS, EXP_COLS)
    if key not in _compiled:
        _compiled[key] = _build_nc(*key)
    nc = _compiled[key]

    qmat, cshift = _make_consts(query)

    in_maps = []
    for c in range(N_CORES):
        e_core = embeddings[c * N_PER_CORE:(c + 1) * N_PER_CORE]
        in_maps.append({
            "epack": _pack_core(e_core),
            "qmat": qmat,
            "cshift": cshift,
        })

    res = None
    for attempt in range(3):
        try:
            res = run_bass_kernel_spmd(nc, in_maps, list(range(N_CORES)))
            break
        except Exception:
            if attempt == 2:
                raise

    wsum = np.zeros(D, dtype=np.float64)
    sumexp = 0.0
    for r in res.results:
        out = r["out"].astype(np.float64)
        wsum += out[0:64, 0] + out[64:128, 0]
        sumexp += out[0, 1] + out[64, 1]
    return (wsum / sumexp).astype(np.float32)
